# revision 34
# baseline (speedup 1.0000x reference)
"""JPEG encoder Bass kernel for TRN2 — self-contained, 8-core data-parallel.

kernel(img, D, Q) -> (flatten, no_quan_flatten), matching the reference:
    per 8x8 block: dct = D @ (X - 128) @ D.T ; quant = round(dct / Q);
    both zigzag-gathered + channel-concatenated to (256, 512, 192).

Design: the per-block pipeline is one linear map on the flattened 64-pixel
block, folded into fp16 matmuls with matrix M = kron(D, D)[zigzag, :].
quant is derived on the fly as int8(rne(nq * 1/Q)) — the f32->int8 output
cast rounds to nearest even, matching jnp.round (|quant| <= 127 here).

Dataflow per core (64 batches), per 2-block-row strip bp (8 iters):
  fp16 strip DMA (6KB rows) -> gpsimd regroup to block-contiguous layout ->
  128x128 PE transposes moving both block rows at once via a block-diag
  identity (3 per block-column pair) -> PSUM->SBUF copies with the -128
  bias fused (scalar + vector engines) -> two fp16 matmuls per block-column
  (K=128 c0|c1 + K=64 c2, N=192) into 256-aligned slots of 2-bank PSUM
  tiles (ring of 3) -> bf16 copy (nq, scalar) / int8 multiply-round-cast
  (q, vector) -> fat contiguous output DMAs (3KB runs per partition).

Schedule notes (cost-model driven): PE seq decode is 71ns/instruction so
transposes are maximally batched (192 + 256 matmuls total); all engine
queues are in-order, so input DMAs prefetch one strip ahead of the output
DMAs on SP, nq DMAs issue from the scalar queue, and the next strip's
transpose stage is emitted split around the current strip's compute halves
to keep every queue free of head-of-line stalls.
"""

import numpy as np
import concourse.mybir as mybir
import concourse.tile as tile
from concourse import bacc
from concourse.bass_utils import run_bass_kernel_spmd

F32 = mybir.dt.float32
F32R = mybir.dt.float32r
F16 = mybir.dt.float16
BF16 = mybir.dt.bfloat16
I8 = mybir.dt.int8
P = 8
B, C, H, W = 512, 3, 128, 128
NCORES = 8
BSH = B // NCORES          # 64 batches per core
N = (H // P) * (W // P)    # 256 blocks per plane
CZ = C * P * P             # 192
NBR = H // P               # 16 block rows
NBW = W // P               # 16 block cols


def _zigzag_flat_idx(n=P):
    order = []
    for s in range(2 * n - 1):
        cells = [(r, s - r) for r in range(max(0, s - n + 1), min(s, n - 1) + 1)]
        if s % 2 == 0:
            cells.reverse()
        order.extend(cells)
    return np.array([r * n + c for r, c in order], dtype=np.int32)


def _build_rhs(D: np.ndarray, Q: np.ndarray):
    ZZ = _zigzag_flat_idx()
    D64 = D.astype(np.float64)
    KD = np.kron(D64, D64)[ZZ, :]          # (64 zz, 64 pix)
    q_zz = Q.astype(np.float64).flatten()[ZZ]
    KDq = KD / q_zz[:, None]
    Mt = KD.T.astype(np.float16)           # (64 pix, 64 zz)
    rhs01 = np.zeros((128, 192), dtype=np.float16)
    for c in (0, 1):
        p0 = c * 64
        rhs01[p0:p0 + 64, c * 64:(c + 1) * 64] = Mt
    # c2 weights on both partition halves: odd block-columns read the c2
    # pixels from partition base 64 (lhsT/rhs partition bases must match)
    rhs2 = np.zeros((128, 192), dtype=np.float16)
    for p0 in (0, 64):
        rhs2[p0:p0 + 64, 128:192] = Mt
    # per-(c,zz) quant reciprocals, repeated for 4 block-columns per PSUM tile
    invq = np.tile((1.0 / q_zz).astype(np.float32), 3)        # (192,)
    invq768 = np.tile(invq, (128, 4))                          # (128, 768)
    return rhs01, rhs2, invq768


def _build_nc():
    nc = bacc.Bacc("TRN2", target_bir_lowering=False, debug=False)

    img = nc.dram_tensor("img", [BSH, C, H, W], F16, kind="ExternalInput")
    rhs01 = nc.dram_tensor("rhs01", [128, 192], F16, kind="ExternalInput")
    rhs2 = nc.dram_tensor("rhs2", [128, 192], F16, kind="ExternalInput")
    invq = nc.dram_tensor("invq", [128, 768], F32, kind="ExternalInput")
    bdid = nc.dram_tensor("bdid", [128, 128], F16, kind="ExternalInput")
    # device layout: [br, b, (bw, c, zz)]; host reassembles to (N, B, CZ)
    nqd = nc.dram_tensor("nqd", [NBR, BSH, 3072], BF16, kind="ExternalOutput")
    qqd = nc.dram_tensor("qqd", [NBR, BSH, 3072], I8, kind="ExternalOutput")

    AddOp = mybir.AluOpType.add
    MulOp = mybir.AluOpType.mult
    Copy = mybir.ActivationFunctionType.Copy

    # partition = b, free = (c, (i, w)) with 2KB contiguous fp16 rows
    imgv = img[:].rearrange(
        "b c (bp brp i) w -> bp brp b c (i w)", brp=2, i=P
    )

    with tile.TileContext(nc) as tc:
        with (
            tc.tile_pool(name="const", bufs=1) as constp,
            tc.tile_pool(name="sload", bufs=4) as sload,
            tc.tile_pool(name="greg", bufs=4) as greg,
            tc.tile_pool(name="xa", bufs=2) as xap,
            tc.tile_pool(name="xb", bufs=2) as xbp,
            tc.tile_pool(name="outs", bufs=2) as outp,
            tc.tile_pool(name="px", bufs=2, space="PSUM") as pxp,
            tc.tile_pool(name="pso", bufs=3, space="PSUM") as psop,
        ):
            r01r = constp.tile([128, 192], F16)
            r2r = constp.tile([128, 192], F16)
            ivq = constp.tile([128, 768], F32)
            bdf = constp.tile([128, 128], F16)

            def load_s(bp, fine=False):
                S = sload.tile([128, 3072], F16, tag="s", name=f"s{bp}")
                if fine:
                    for c in range(3):
                        for brp in range(2):
                            nc.sync.dma_start(
                                out=S[brp * 64:(brp + 1) * 64,
                                      c * 1024:(c + 1) * 1024],
                                in_=imgv[bp, brp, :, c],
                            )
                else:
                    for brp in range(2):
                        nc.sync.dma_start(
                            out=S[brp * 64:(brp + 1) * 64, :],
                            in_=imgv[bp, brp],
                        )
                return S

            def regroup(S, bp, dve=False):
                # regroup on the gpsimd engine into per-block-column-pair
                # groups of 6 x 64 pixels: z = (k*2+c) for c0/c1, 4+k for c2
                # (bw = 2q+k); G free offset = q*384 + z*64 + (i*8+j)
                G = greg.tile([128, 3072], F16, tag="g", name=f"g{bp}")
                gv = G[:].rearrange("p (q z i j) -> p z q i j",
                                    q=8, z=6, i=P, j=P)
                sv = S[:].rearrange("p (c i q k j) -> p c k q i j",
                                    c=3, i=P, q=8, k=2, j=P)
                for c, k in ((0, 0), (1, 0), (0, 1), (1, 1), (2, 0), (2, 1)):
                    z = 4 + k if c == 2 else k * 2 + c
                    if dve:
                        nc.vector.tensor_copy(gv[:, z], sv[:, c, k])
                    else:
                        nc.gpsimd.tensor_copy(gv[:, z], sv[:, c, k])
                return G

            def transpose_part1(G, bp):
                # k0 transposes + their scalar-engine copy; emitted before
                # the previous strip's compute so xA fills early
                xA = xap.tile([128, 2048], F16, tag="xa", name=f"xa{bp}")
                xB = xbp.tile([128, 1024], F16, tag="xb", name=f"xb{bp}")
                px = pxp.tile([128, 1024], F16, tag="px", name=f"pxa{bp}k0")
                for q in range(8):
                    nc.tensor.matmul(
                        px[:, q * 128:(q + 1) * 128],
                        G[:, q * 384:q * 384 + 128],
                        bdf[:], is_transpose=True,
                    )
                nc.scalar.activation(
                    xA[:].rearrange("p (q f) -> p q f", q=16)[:, 0::2],
                    px[:].rearrange("p (q f) -> p q f", q=8),
                    Copy, bias=-128.0,
                )
                return xA, xB

            def transpose_part2(G, xA, xB, bp):
                # k1 + c2 transposes; their vector-engine copies land after
                # the previous strip's g0/g1 q-copies in the DVE queue
                px = pxp.tile([128, 1024], F16, tag="px", name=f"pxa{bp}k1")
                for q in range(8):
                    nc.tensor.matmul(
                        px[:, q * 128:(q + 1) * 128],
                        G[:, q * 384 + 128:q * 384 + 256],
                        bdf[:], is_transpose=True,
                    )
                nc.vector.tensor_scalar(
                    xA[:].rearrange("p (q f) -> p q f", q=16)[:, 1::2],
                    px[:].rearrange("p (q f) -> p q f", q=8),
                    -128.0, None, AddOp,
                )
                px = pxp.tile([128, 1024], F16, tag="px", name=f"pxb{bp}")
                for q in range(8):
                    nc.tensor.matmul(
                        px[:, q * 128:(q + 1) * 128],
                        G[:, q * 384 + 256:q * 384 + 384],
                        bdf[:], is_transpose=True,
                    )
                nc.vector.tensor_scalar(xB[:], px[:], -128.0, None, AddOp)

            def compute_half(xA, xB, bp, h, stnq, stq):
                for g in (h * 2, h * 2 + 1):  # 4 block-columns per PSUM tile
                    po = psop.tile([128, 1024], F32)
                    for s in range(4):
                        bw = g * 4 + s
                        q, k = bw // 2, bw % 2
                        nc.tensor.matmul(
                            po[:, s * 256:s * 256 + 192],
                            xA[:, bw * 128:(bw + 1) * 128], r01r[:],
                            start=True, stop=False,
                        )
                        nc.tensor.matmul(
                            po[:, s * 256:s * 256 + 192],
                            xB[k * 64:(k + 1) * 64, q * 128:(q + 1) * 128],
                            r2r[k * 64:(k + 1) * 64, :],
                            start=False, stop=True,
                        )
                    pov = po[:].rearrange("p (s f) -> p s f", s=4)[:, :, 0:192]
                    nc.scalar.activation(
                        stnq[:, g * 768:(g + 1) * 768].rearrange(
                            "p (s f) -> p s f", s=4),
                        pov, Copy,
                    )
                    # q = rne_int8(nq * (1/Q)): (po * 1.0) * invq, cast int8
                    nc.vector.scalar_tensor_tensor(
                        stq[:, g * 768:(g + 1) * 768].rearrange(
                            "p (s f) -> p s f", s=4),
                        pov, 1.0,
                        ivq[:].rearrange("p (s f) -> p s f", s=4),
                        MulOp, MulOp,
                    )

            def out_dmas(bp, stnq, stq):
                qhalves = 2 if bp == 7 else 1
                for brp in range(2):
                    br = bp * 2 + brp
                    bsl = slice(brp * 64, (brp + 1) * 64)
                    for h in range(2):
                        fsl = slice(h * 1536, (h + 1) * 1536)
                        # nq out-DMA issues from the scalar queue (its
                        # producer); q out-DMA stays on SP, behind the
                        # next-strip prefetch
                        nc.scalar.dma_start(out=nqd[br, :, fsl],
                                            in_=stnq[bsl, fsl])
                    for h in range(qhalves):
                        fh = 3072 // qhalves
                        fsl = slice(h * fh, (h + 1) * fh)
                        nc.sync.dma_start(out=qqd[br, :, fsl],
                                          in_=stq[bsl, fsl])

            # software-pipelined emission: transposes/copies of bp+1 are
            # emitted BEFORE the matmul/output phase of bp so no engine
            # queue has head-of-line stalls at strip boundaries
            S_cur = load_s(0)
            # constants issue behind the first strip's loads; the first
            # regroup runs on the (idle at start) vector engine
            nc.sync.dma_start(out=bdf[:], in_=bdid[:])
            nc.sync.dma_start(out=r01r[:], in_=rhs01[:])
            nc.sync.dma_start(out=r2r[:], in_=rhs2[:])
            nc.sync.dma_start(out=ivq[:], in_=invq[:])
            G_cur = regroup(S_cur, 0, dve=True)
            S_nxt = load_s(1)
            x_cur = transpose_part1(G_cur, 0)
            transpose_part2(G_cur, x_cur[0], x_cur[1], 0)
            for bp in range(8):
                stnq = outp.tile([128, 3072], BF16, tag="stnq", name="stnq")
                stq = outp.tile([128, 3072], I8, tag="stq", name="stq")
                last = bp + 1 >= 8
                if not last:
                    G_nxt = regroup(S_nxt, bp + 1)
                    if bp + 2 < 8:
                        S_nxt = load_s(bp + 2)
                    x_nxt = transpose_part1(G_nxt, bp + 1)
                compute_half(x_cur[0], x_cur[1], bp, 0, stnq, stq)
                if not last:
                    transpose_part2(G_nxt, x_nxt[0], x_nxt[1], bp + 1)
                compute_half(x_cur[0], x_cur[1], bp, 1, stnq, stq)
                out_dmas(bp, stnq, stq)
                if not last:
                    x_cur = x_nxt

    nc.compile()
    return nc


_NC_CACHE = None


def _get_nc():
    global _NC_CACHE
    if _NC_CACHE is None:
        _NC_CACHE = _build_nc()
    return _NC_CACHE


def _unshard(dev_out: np.ndarray) -> np.ndarray:
    # [br, b, (bw c zz)] -> (N, BSH, CZ) with n = br*16 + bw
    a = dev_out.reshape(NBR, BSH, NBW, CZ).astype(np.float32)
    return a.transpose(0, 2, 1, 3).reshape(N, BSH, CZ)


def kernel(img, D, Q):
    img = np.ascontiguousarray(np.asarray(img, dtype=np.float32))
    D = np.asarray(D, dtype=np.float32)
    Q = np.asarray(Q, dtype=np.float32)
    rhs01, rhs2, invq768 = _build_rhs(D, Q)
    e64 = np.eye(64, dtype=np.float16)
    z64 = np.zeros((64, 64), dtype=np.float16)
    bdid = np.block([[e64, z64], [z64, e64]]).astype(np.float16)

    img16 = img.astype(np.float16)
    nc = _get_nc()
    in_maps = [
        {"img": img16[k * BSH:(k + 1) * BSH], "rhs01": rhs01, "rhs2": rhs2,
         "bdid": bdid, "invq": invq768}
        for k in range(NCORES)
    ]
    res = run_bass_kernel_spmd(nc, in_maps, core_ids=list(range(NCORES)))
    flatten = np.concatenate(
        [_unshard(np.asarray(r["qqd"])) for r in res.results], axis=1)
    no_quan = np.concatenate(
        [_unshard(np.asarray(r["nqd"])) for r in res.results], axis=1)
    return (flatten, no_quan)


# revision 35
# speedup vs baseline: 1.0380x; 1.0380x over previous
"""JPEG encoder Bass kernel for TRN2 — self-contained, 8-core data-parallel.

kernel(img, D, Q) -> (flatten, no_quan_flatten), matching the reference:
    per 8x8 block: dct = D @ (X - 128) @ D.T ; quant = round(dct / Q);
    both zigzag-gathered + channel-concatenated to (256, 512, 192).

Design: the per-block pipeline is one linear map on the flattened 64-pixel
block, folded into fp16 matmuls with matrix M = kron(D, D)[zigzag, :].
quant is derived on the fly as int8(rne(nq * 1/Q)) — the f32->int8 output
cast rounds to nearest even, matching jnp.round (|quant| <= 127 here).

Dataflow per core (64 batches), per 2-block-row strip bp (8 iters):
  fp16 strip DMA (6KB rows) -> gpsimd regroup to block-contiguous layout ->
  128x128 PE transposes moving both block rows at once via a block-diag
  identity (3 per block-column pair) -> PSUM->SBUF copies with the -128
  bias fused (scalar + vector engines) -> two fp16 matmuls per block-column
  (K=128 c0|c1 + K=64 c2, N=192) into 256-aligned slots of 2-bank PSUM
  tiles (ring of 3) -> bf16 copy (nq, scalar) / int8 multiply-round-cast
  (q, vector) -> fat contiguous output DMAs (3KB runs per partition).

Schedule notes (cost-model driven): PE seq decode is 71ns/instruction so
transposes are maximally batched (192 + 256 matmuls total); all engine
queues are in-order, so input DMAs prefetch one strip ahead of the output
DMAs on SP, nq DMAs issue from the scalar queue, and the next strip's
transpose stage is emitted split around the current strip's compute halves
to keep every queue free of head-of-line stalls.
"""

import numpy as np
import concourse.mybir as mybir
import concourse.tile as tile
from concourse import bacc
from concourse.bass_utils import run_bass_kernel_spmd

F32 = mybir.dt.float32
F32R = mybir.dt.float32r
F16 = mybir.dt.float16
BF16 = mybir.dt.bfloat16
I8 = mybir.dt.int8
P = 8
B, C, H, W = 512, 3, 128, 128
NCORES = 8
BSH = B // NCORES          # 64 batches per core
N = (H // P) * (W // P)    # 256 blocks per plane
CZ = C * P * P             # 192
NBR = H // P               # 16 block rows
NBW = W // P               # 16 block cols


def _zigzag_flat_idx(n=P):
    order = []
    for s in range(2 * n - 1):
        cells = [(r, s - r) for r in range(max(0, s - n + 1), min(s, n - 1) + 1)]
        if s % 2 == 0:
            cells.reverse()
        order.extend(cells)
    return np.array([r * n + c for r, c in order], dtype=np.int32)


def _build_rhs(D: np.ndarray, Q: np.ndarray):
    ZZ = _zigzag_flat_idx()
    D64 = D.astype(np.float64)
    KD = np.kron(D64, D64)[ZZ, :]          # (64 zz, 64 pix)
    q_zz = Q.astype(np.float64).flatten()[ZZ]
    KDq = KD / q_zz[:, None]
    Mt = KD.T.astype(np.float16)           # (64 pix, 64 zz)
    rhs01 = np.zeros((128, 192), dtype=np.float16)
    for c in (0, 1):
        p0 = c * 64
        rhs01[p0:p0 + 64, c * 64:(c + 1) * 64] = Mt
    # c2 weights on both partition halves: odd block-columns read the c2
    # pixels from partition base 64 (lhsT/rhs partition bases must match)
    rhs2 = np.zeros((128, 192), dtype=np.float16)
    for p0 in (0, 64):
        rhs2[p0:p0 + 64, 128:192] = Mt
    # per-(c,zz) quant reciprocals, repeated for 4 block-columns per PSUM tile
    invq = np.tile((1.0 / q_zz).astype(np.float32), 3)        # (192,)
    invq768 = np.tile(invq, (128, 4))                          # (128, 768)
    return rhs01, rhs2, invq768


def _build_nc():
    nc = bacc.Bacc("TRN2", target_bir_lowering=False, debug=False)

    img = nc.dram_tensor("img", [BSH, C, H, W], F16, kind="ExternalInput")
    rhs01 = nc.dram_tensor("rhs01", [128, 192], F16, kind="ExternalInput")
    rhs2 = nc.dram_tensor("rhs2", [128, 192], F16, kind="ExternalInput")
    invq = nc.dram_tensor("invq", [128, 768], F32, kind="ExternalInput")
    bdid = nc.dram_tensor("bdid", [128, 128], F16, kind="ExternalInput")
    # device layout: [br, b, (bw, c, zz)]; host reassembles to (N, B, CZ)
    nqd = nc.dram_tensor("nqd", [NBR, BSH, 3072], BF16, kind="ExternalOutput")
    qqd = nc.dram_tensor("qqd", [NBR, BSH, 3072], I8, kind="ExternalOutput")

    AddOp = mybir.AluOpType.add
    MulOp = mybir.AluOpType.mult
    Copy = mybir.ActivationFunctionType.Copy

    # partition = b, free = (c, (i, w)) with 2KB contiguous fp16 rows
    imgv = img[:].rearrange(
        "b c (bp brp i) w -> bp brp b c (i w)", brp=2, i=P
    )

    with tile.TileContext(nc) as tc:
        with (
            tc.tile_pool(name="const", bufs=1) as constp,
            tc.tile_pool(name="sload", bufs=4) as sload,
            tc.tile_pool(name="greg", bufs=4) as greg,
            tc.tile_pool(name="xa", bufs=2) as xap,
            tc.tile_pool(name="xb", bufs=2) as xbp,
            tc.tile_pool(name="outs", bufs=2) as outp,
            tc.tile_pool(name="px", bufs=2, space="PSUM") as pxp,
            tc.tile_pool(name="pso", bufs=3, space="PSUM") as psop,
        ):
            r01r = constp.tile([128, 192], F16)
            r2r = constp.tile([128, 192], F16)
            ivq = constp.tile([128, 768], F32)
            bdf = constp.tile([128, 128], F16)

            def load_s(bp, fine=False):
                S = sload.tile([128, 3072], F16, tag="s", name=f"s{bp}")
                if fine:
                    for c in range(3):
                        for brp in range(2):
                            nc.sync.dma_start(
                                out=S[brp * 64:(brp + 1) * 64,
                                      c * 1024:(c + 1) * 1024],
                                in_=imgv[bp, brp, :, c],
                            )
                else:
                    # one DMA per strip: 4D src AP pairs with the 2D dst in
                    # iteration order (brp, b, c, iw) == (partition, free)
                    nc.sync.dma_start(out=S[:], in_=imgv[bp])
                return S

            def regroup(S, bp, dve=False):
                # regroup on the gpsimd engine into per-block-column-pair
                # groups of 6 x 64 pixels: z = (k*2+c) for c0/c1, 4+k for c2
                # (bw = 2q+k); G free offset = q*384 + z*64 + (i*8+j)
                G = greg.tile([128, 3072], F16, tag="g", name=f"g{bp}")
                gv = G[:].rearrange("p (q z i j) -> p z q i j",
                                    q=8, z=6, i=P, j=P)
                sv = S[:].rearrange("p (c i q k j) -> p c k q i j",
                                    c=3, i=P, q=8, k=2, j=P)
                for c, k in ((0, 0), (1, 0), (0, 1), (1, 1), (2, 0), (2, 1)):
                    z = 4 + k if c == 2 else k * 2 + c
                    if dve:
                        nc.vector.tensor_copy(gv[:, z], sv[:, c, k])
                    else:
                        nc.gpsimd.tensor_copy(gv[:, z], sv[:, c, k])
                return G

            def transpose_part1(G, bp):
                # k0 transposes + their scalar-engine copy; emitted before
                # the previous strip's compute so xA fills early
                xA = xap.tile([128, 2048], F16, tag="xa", name=f"xa{bp}")
                xB = xbp.tile([128, 1024], F16, tag="xb", name=f"xb{bp}")
                px = pxp.tile([128, 1024], F16, tag="px", name=f"pxa{bp}k0")
                for q in range(8):
                    nc.tensor.matmul(
                        px[:, q * 128:(q + 1) * 128],
                        G[:, q * 384:q * 384 + 128],
                        bdf[:], is_transpose=True,
                    )
                nc.scalar.activation(
                    xA[:].rearrange("p (q f) -> p q f", q=16)[:, 0::2],
                    px[:].rearrange("p (q f) -> p q f", q=8),
                    Copy, bias=-128.0,
                )
                return xA, xB

            def transpose_part2(G, xA, xB, bp):
                # k1 + c2 transposes; their vector-engine copies land after
                # the previous strip's g0/g1 q-copies in the DVE queue
                px = pxp.tile([128, 1024], F16, tag="px", name=f"pxa{bp}k1")
                for q in range(8):
                    nc.tensor.matmul(
                        px[:, q * 128:(q + 1) * 128],
                        G[:, q * 384 + 128:q * 384 + 256],
                        bdf[:], is_transpose=True,
                    )
                nc.vector.tensor_scalar(
                    xA[:].rearrange("p (q f) -> p q f", q=16)[:, 1::2],
                    px[:].rearrange("p (q f) -> p q f", q=8),
                    -128.0, None, AddOp,
                )
                px = pxp.tile([128, 1024], F16, tag="px", name=f"pxb{bp}")
                for q in range(8):
                    nc.tensor.matmul(
                        px[:, q * 128:(q + 1) * 128],
                        G[:, q * 384 + 256:q * 384 + 384],
                        bdf[:], is_transpose=True,
                    )
                nc.vector.tensor_scalar(xB[:], px[:], -128.0, None, AddOp)

            def compute_half(xA, xB, bp, h, stnq, stq):
                for g in (h * 2, h * 2 + 1):  # 4 block-columns per PSUM tile
                    po = psop.tile([128, 1024], F32)
                    for s in range(4):
                        bw = g * 4 + s
                        q, k = bw // 2, bw % 2
                        nc.tensor.matmul(
                            po[:, s * 256:s * 256 + 192],
                            xA[:, bw * 128:(bw + 1) * 128], r01r[:],
                            start=True, stop=False,
                        )
                        nc.tensor.matmul(
                            po[:, s * 256:s * 256 + 192],
                            xB[k * 64:(k + 1) * 64, q * 128:(q + 1) * 128],
                            r2r[k * 64:(k + 1) * 64, :],
                            start=False, stop=True,
                        )
                    pov = po[:].rearrange("p (s f) -> p s f", s=4)[:, :, 0:192]
                    nc.scalar.activation(
                        stnq[:, g * 768:(g + 1) * 768].rearrange(
                            "p (s f) -> p s f", s=4),
                        pov, Copy,
                    )
                    # q = rne_int8(nq * (1/Q)): (po * 1.0) * invq, cast int8
                    nc.vector.scalar_tensor_tensor(
                        stq[:, g * 768:(g + 1) * 768].rearrange(
                            "p (s f) -> p s f", s=4),
                        pov, 1.0,
                        ivq[:].rearrange("p (s f) -> p s f", s=4),
                        MulOp, MulOp,
                    )

            def out_dmas(bp, stnq, stq):
                qhalves = 2 if bp == 7 else 1
                for brp in range(2):
                    br = bp * 2 + brp
                    bsl = slice(brp * 64, (brp + 1) * 64)
                    for h in range(2):
                        fsl = slice(h * 1536, (h + 1) * 1536)
                        # nq out-DMA issues from the scalar queue (its
                        # producer); q out-DMA stays on SP, behind the
                        # next-strip prefetch
                        nc.scalar.dma_start(out=nqd[br, :, fsl],
                                            in_=stnq[bsl, fsl])
                    for h in range(qhalves):
                        fh = 3072 // qhalves
                        fsl = slice(h * fh, (h + 1) * fh)
                        nc.sync.dma_start(out=qqd[br, :, fsl],
                                          in_=stq[bsl, fsl])

            # software-pipelined emission: transposes/copies of bp+1 are
            # emitted BEFORE the matmul/output phase of bp so no engine
            # queue has head-of-line stalls at strip boundaries
            S_cur = load_s(0)
            # constants issue behind the first strip's loads; the first
            # regroup runs on the (idle at start) vector engine
            nc.sync.dma_start(out=bdf[:], in_=bdid[:])
            nc.sync.dma_start(out=r01r[:], in_=rhs01[:])
            nc.sync.dma_start(out=r2r[:], in_=rhs2[:])
            nc.sync.dma_start(out=ivq[:], in_=invq[:])
            G_cur = regroup(S_cur, 0, dve=True)
            S_nxt = load_s(1)
            x_cur = transpose_part1(G_cur, 0)
            transpose_part2(G_cur, x_cur[0], x_cur[1], 0)
            for bp in range(8):
                stnq = outp.tile([128, 3072], BF16, tag="stnq", name="stnq")
                stq = outp.tile([128, 3072], I8, tag="stq", name="stq")
                last = bp + 1 >= 8
                if not last:
                    G_nxt = regroup(S_nxt, bp + 1)
                    if bp + 2 < 8:
                        S_nxt = load_s(bp + 2)
                    x_nxt = transpose_part1(G_nxt, bp + 1)
                compute_half(x_cur[0], x_cur[1], bp, 0, stnq, stq)
                if not last:
                    transpose_part2(G_nxt, x_nxt[0], x_nxt[1], bp + 1)
                compute_half(x_cur[0], x_cur[1], bp, 1, stnq, stq)
                out_dmas(bp, stnq, stq)
                if not last:
                    x_cur = x_nxt

    nc.compile()
    return nc


_NC_CACHE = None


def _get_nc():
    global _NC_CACHE
    if _NC_CACHE is None:
        _NC_CACHE = _build_nc()
    return _NC_CACHE


def _unshard(dev_out: np.ndarray) -> np.ndarray:
    # [br, b, (bw c zz)] -> (N, BSH, CZ) with n = br*16 + bw
    a = dev_out.reshape(NBR, BSH, NBW, CZ).astype(np.float32)
    return a.transpose(0, 2, 1, 3).reshape(N, BSH, CZ)


def kernel(img, D, Q):
    img = np.ascontiguousarray(np.asarray(img, dtype=np.float32))
    D = np.asarray(D, dtype=np.float32)
    Q = np.asarray(Q, dtype=np.float32)
    rhs01, rhs2, invq768 = _build_rhs(D, Q)
    e64 = np.eye(64, dtype=np.float16)
    z64 = np.zeros((64, 64), dtype=np.float16)
    bdid = np.block([[e64, z64], [z64, e64]]).astype(np.float16)

    img16 = img.astype(np.float16)
    nc = _get_nc()
    in_maps = [
        {"img": img16[k * BSH:(k + 1) * BSH], "rhs01": rhs01, "rhs2": rhs2,
         "bdid": bdid, "invq": invq768}
        for k in range(NCORES)
    ]
    res = run_bass_kernel_spmd(nc, in_maps, core_ids=list(range(NCORES)))
    flatten = np.concatenate(
        [_unshard(np.asarray(r["qqd"])) for r in res.results], axis=1)
    no_quan = np.concatenate(
        [_unshard(np.asarray(r["nqd"])) for r in res.results], axis=1)
    return (flatten, no_quan)


# revision 37
# speedup vs baseline: 1.0529x; 1.0143x over previous
"""JPEG encoder Bass kernel for TRN2 — self-contained, 8-core data-parallel.

kernel(img, D, Q) -> (flatten, no_quan_flatten), matching the reference:
    per 8x8 block: dct = D @ (X - 128) @ D.T ; quant = round(dct / Q);
    both zigzag-gathered + channel-concatenated to (256, 512, 192).

Design: the per-block pipeline is one linear map on the flattened 64-pixel
block, folded into fp16 matmuls with matrix M = kron(D, D)[zigzag, :].
quant is derived on the fly as int8(rne(nq * 1/Q)) — the f32->int8 output
cast rounds to nearest even, matching jnp.round (|quant| <= 127 here).

Dataflow per core (64 batches), per 2-block-row strip bp (8 iters):
  fp16 strip DMA (6KB rows) -> gpsimd regroup to block-contiguous layout ->
  128x128 PE transposes moving both block rows at once via a block-diag
  identity (3 per block-column pair) -> PSUM->SBUF copies with the -128
  bias fused (scalar + vector engines) -> two fp16 matmuls per block-column
  (K=128 c0|c1 + K=64 c2, N=192) into 256-aligned slots of 2-bank PSUM
  tiles (ring of 3) -> bf16 copy (nq, scalar) / int8 multiply-round-cast
  (q, vector) -> fat contiguous output DMAs (3KB runs per partition).

Schedule notes (cost-model driven): PE seq decode is 71ns/instruction so
transposes are maximally batched (192 + 256 matmuls total); all engine
queues are in-order, so input DMAs prefetch one strip ahead of the output
DMAs on SP, nq DMAs issue from the scalar queue, and the next strip's
transpose stage is emitted split around the current strip's compute halves
to keep every queue free of head-of-line stalls.
"""

import numpy as np
import concourse.mybir as mybir
import concourse.tile as tile
from concourse import bacc
from concourse.bass_utils import run_bass_kernel_spmd

F32 = mybir.dt.float32
F32R = mybir.dt.float32r
F16 = mybir.dt.float16
BF16 = mybir.dt.bfloat16
I8 = mybir.dt.int8
P = 8
B, C, H, W = 512, 3, 128, 128
NCORES = 8
BSH = B // NCORES          # 64 batches per core
N = (H // P) * (W // P)    # 256 blocks per plane
CZ = C * P * P             # 192
NBR = H // P               # 16 block rows
NBW = W // P               # 16 block cols


def _zigzag_flat_idx(n=P):
    order = []
    for s in range(2 * n - 1):
        cells = [(r, s - r) for r in range(max(0, s - n + 1), min(s, n - 1) + 1)]
        if s % 2 == 0:
            cells.reverse()
        order.extend(cells)
    return np.array([r * n + c for r, c in order], dtype=np.int32)


def _build_rhs(D: np.ndarray, Q: np.ndarray):
    ZZ = _zigzag_flat_idx()
    D64 = D.astype(np.float64)
    KD = np.kron(D64, D64)[ZZ, :]          # (64 zz, 64 pix)
    q_zz = Q.astype(np.float64).flatten()[ZZ]
    KDq = KD / q_zz[:, None]
    Mt = KD.T.astype(np.float16)           # (64 pix, 64 zz)
    rhs01 = np.zeros((128, 192), dtype=np.float16)
    for c in (0, 1):
        p0 = c * 64
        rhs01[p0:p0 + 64, c * 64:(c + 1) * 64] = Mt
    # c2 weights on both partition halves: odd block-columns read the c2
    # pixels from partition base 64 (lhsT/rhs partition bases must match)
    rhs2 = np.zeros((128, 192), dtype=np.float16)
    for p0 in (0, 64):
        rhs2[p0:p0 + 64, 128:192] = Mt
    # per-(c,zz) quant reciprocals, repeated for 4 block-columns per PSUM tile
    invq = np.tile((1.0 / q_zz).astype(np.float32), 3)        # (192,)
    invq768 = np.tile(invq, (128, 4))                          # (128, 768)
    return rhs01, rhs2, invq768


def _build_nc():
    nc = bacc.Bacc("TRN2", target_bir_lowering=False, debug=False)

    img = nc.dram_tensor("img", [BSH, C, H, W], F16, kind="ExternalInput")
    rhs01 = nc.dram_tensor("rhs01", [128, 192], F16, kind="ExternalInput")
    rhs2 = nc.dram_tensor("rhs2", [128, 192], F16, kind="ExternalInput")
    invq = nc.dram_tensor("invq", [128, 768], F32, kind="ExternalInput")
    bdid = nc.dram_tensor("bdid", [128, 128], F16, kind="ExternalInput")
    # device layout: [br, b, (bw, c, zz)]; host reassembles to (N, B, CZ)
    nqd = nc.dram_tensor("nqd", [NBR, BSH, 3072], BF16, kind="ExternalOutput")
    qqd = nc.dram_tensor("qqd", [NBR, BSH, 3072], I8, kind="ExternalOutput")

    AddOp = mybir.AluOpType.add
    MulOp = mybir.AluOpType.mult
    Copy = mybir.ActivationFunctionType.Copy

    # partition = b, free = (c, (i, w)) with 2KB contiguous fp16 rows
    imgv = img[:].rearrange(
        "b c (bp brp i) w -> bp brp b c (i w)", brp=2, i=P
    )

    with tile.TileContext(nc) as tc:
        with (
            tc.tile_pool(name="const", bufs=1) as constp,
            tc.tile_pool(name="sload", bufs=4) as sload,
            tc.tile_pool(name="greg", bufs=4) as greg,
            tc.tile_pool(name="xa", bufs=2) as xap,
            tc.tile_pool(name="xb", bufs=2) as xbp,
            tc.tile_pool(name="outs", bufs=2) as outp,
            tc.tile_pool(name="px", bufs=2, space="PSUM") as pxp,
            tc.tile_pool(name="pso", bufs=3, space="PSUM") as psop,
        ):
            r01r = constp.tile([128, 192], F16)
            r2r = constp.tile([128, 192], F16)
            ivq = constp.tile([128, 768], F32)
            bdf = constp.tile([128, 128], F16)

            def load_s(bp, fine=False):
                S = sload.tile([128, 3072], F16, tag="s", name=f"s{bp}")
                if fine:
                    for c in range(3):
                        for brp in range(2):
                            nc.sync.dma_start(
                                out=S[brp * 64:(brp + 1) * 64,
                                      c * 1024:(c + 1) * 1024],
                                in_=imgv[bp, brp, :, c],
                            )
                else:
                    # one DMA per strip: 4D src AP pairs with the 2D dst in
                    # iteration order (brp, b, c, iw) == (partition, free)
                    nc.sync.dma_start(out=S[:], in_=imgv[bp])
                return S

            def regroup(S, bp, dve=False):
                # regroup on the gpsimd engine into per-block-column-pair
                # groups of 6 x 64 pixels: z = (k*2+c) for c0/c1, 4+k for c2
                # (bw = 2q+k); G free offset = q*384 + z*64 + (i*8+j)
                G = greg.tile([128, 3072], F16, tag="g", name=f"g{bp}")
                gv = G[:].rearrange("p (q z i j) -> p z q i j",
                                    q=8, z=6, i=P, j=P)
                sv = S[:].rearrange("p (c i q k j) -> p c k q i j",
                                    c=3, i=P, q=8, k=2, j=P)
                for c, k in ((0, 0), (1, 0), (0, 1), (1, 1), (2, 0), (2, 1)):
                    z = 4 + k if c == 2 else k * 2 + c
                    if dve:
                        nc.vector.tensor_copy(gv[:, z], sv[:, c, k])
                    else:
                        nc.gpsimd.tensor_copy(gv[:, z], sv[:, c, k])
                return G

            def transpose_part1(G, bp):
                # k0 transposes + their scalar-engine copy; emitted before
                # the previous strip's compute so xA fills early
                xA = xap.tile([128, 2048], F16, tag="xa", name=f"xa{bp}")
                xB = xbp.tile([128, 1024], F16, tag="xb", name=f"xb{bp}")
                px = pxp.tile([128, 1024], F16, tag="px", name=f"pxa{bp}k0")
                for q in range(8):
                    nc.tensor.matmul(
                        px[:, q * 128:(q + 1) * 128],
                        G[:, q * 384:q * 384 + 128],
                        bdf[:], is_transpose=True,
                    )
                nc.scalar.activation(
                    xA[:].rearrange("p (q f) -> p q f", q=16)[:, 0::2],
                    px[:].rearrange("p (q f) -> p q f", q=8),
                    Copy, bias=-128.0,
                )
                return xA, xB

            def transpose_part2(G, xA, xB, bp):
                # k1 + c2 transposes; their vector-engine copies land after
                # the previous strip's g0/g1 q-copies in the DVE queue
                px = pxp.tile([128, 1024], F16, tag="px", name=f"pxa{bp}k1")
                for q in range(8):
                    nc.tensor.matmul(
                        px[:, q * 128:(q + 1) * 128],
                        G[:, q * 384 + 128:q * 384 + 256],
                        bdf[:], is_transpose=True,
                    )
                nc.vector.tensor_scalar(
                    xA[:].rearrange("p (q f) -> p q f", q=16)[:, 1::2],
                    px[:].rearrange("p (q f) -> p q f", q=8),
                    -128.0, None, AddOp,
                )
                px = pxp.tile([128, 1024], F16, tag="px", name=f"pxb{bp}")
                for q in range(8):
                    nc.tensor.matmul(
                        px[:, q * 128:(q + 1) * 128],
                        G[:, q * 384 + 256:q * 384 + 384],
                        bdf[:], is_transpose=True,
                    )
                nc.vector.tensor_scalar(xB[:], px[:], -128.0, None, AddOp)

            def compute_half(xA, xB, bp, h, stnq, stq):
                for g in (h * 2, h * 2 + 1):  # 4 block-columns per PSUM tile
                    po = psop.tile([128, 1024], F32)
                    for s in range(4):
                        bw = g * 4 + s
                        q, k = bw // 2, bw % 2
                        nc.tensor.matmul(
                            po[:, s * 256:s * 256 + 192],
                            xA[:, bw * 128:(bw + 1) * 128], r01r[:],
                            start=True, stop=False,
                        )
                        nc.tensor.matmul(
                            po[:, s * 256:s * 256 + 192],
                            xB[k * 64:(k + 1) * 64, q * 128:(q + 1) * 128],
                            r2r[k * 64:(k + 1) * 64, :],
                            start=False, stop=True,
                        )
                    pov = po[:].rearrange("p (s f) -> p s f", s=4)[:, :, 0:192]
                    nc.scalar.activation(
                        stnq[:, g * 768:(g + 1) * 768].rearrange(
                            "p (s f) -> p s f", s=4),
                        pov, Copy,
                    )
                    # q = rne_int8(nq * (1/Q)): (po * 1.0) * invq, cast int8
                    nc.vector.scalar_tensor_tensor(
                        stq[:, g * 768:(g + 1) * 768].rearrange(
                            "p (s f) -> p s f", s=4),
                        pov, 1.0,
                        ivq[:].rearrange("p (s f) -> p s f", s=4),
                        MulOp, MulOp,
                    )

            def out_dmas(bp, stnq, stq):
                qhalves = 2 if bp == 7 else 1
                for brp in range(2):
                    br = bp * 2 + brp
                    bsl = slice(brp * 64, (brp + 1) * 64)
                    for h in range(2):
                        fsl = slice(h * 1536, (h + 1) * 1536)
                        # nq out-DMA issues from the scalar queue (its
                        # producer); q out-DMA stays on SP, behind the
                        # next-strip prefetch
                        nc.scalar.dma_start(out=nqd[br, :, fsl],
                                            in_=stnq[bsl, fsl])
                    for h in range(qhalves):
                        fh = 3072 // qhalves
                        fsl = slice(h * fh, (h + 1) * fh)
                        nc.sync.dma_start(out=qqd[br, :, fsl],
                                          in_=stq[bsl, fsl])

            # software-pipelined emission: transposes/copies of bp+1 are
            # emitted BEFORE the matmul/output phase of bp so no engine
            # queue has head-of-line stalls at strip boundaries
            S_cur = load_s(0)
            # constants issue behind the first strip's loads; the first
            # regroup runs on the (idle at start) vector engine
            nc.scalar.dma_start(out=bdf[:], in_=bdid[:])
            nc.scalar.dma_start(out=r01r[:], in_=rhs01[:])
            nc.scalar.dma_start(out=r2r[:], in_=rhs2[:])
            nc.scalar.dma_start(out=ivq[:], in_=invq[:])
            G_cur = regroup(S_cur, 0, dve=True)
            S_nxt = load_s(1)
            x_cur = transpose_part1(G_cur, 0)
            transpose_part2(G_cur, x_cur[0], x_cur[1], 0)
            for bp in range(8):
                stnq = outp.tile([128, 3072], BF16, tag="stnq", name="stnq")
                stq = outp.tile([128, 3072], I8, tag="stq", name="stq")
                last = bp + 1 >= 8
                if not last:
                    G_nxt = regroup(S_nxt, bp + 1)
                    if bp + 2 < 8:
                        S_nxt = load_s(bp + 2)
                    x_nxt = transpose_part1(G_nxt, bp + 1)
                compute_half(x_cur[0], x_cur[1], bp, 0, stnq, stq)
                if not last:
                    transpose_part2(G_nxt, x_nxt[0], x_nxt[1], bp + 1)
                compute_half(x_cur[0], x_cur[1], bp, 1, stnq, stq)
                out_dmas(bp, stnq, stq)
                if not last:
                    x_cur = x_nxt

    nc.compile()
    return nc


_NC_CACHE = None


def _get_nc():
    global _NC_CACHE
    if _NC_CACHE is None:
        _NC_CACHE = _build_nc()
    return _NC_CACHE


def _unshard(dev_out: np.ndarray) -> np.ndarray:
    # [br, b, (bw c zz)] -> (N, BSH, CZ) with n = br*16 + bw
    a = dev_out.reshape(NBR, BSH, NBW, CZ).astype(np.float32)
    return a.transpose(0, 2, 1, 3).reshape(N, BSH, CZ)


def kernel(img, D, Q):
    img = np.ascontiguousarray(np.asarray(img, dtype=np.float32))
    D = np.asarray(D, dtype=np.float32)
    Q = np.asarray(Q, dtype=np.float32)
    rhs01, rhs2, invq768 = _build_rhs(D, Q)
    e64 = np.eye(64, dtype=np.float16)
    z64 = np.zeros((64, 64), dtype=np.float16)
    bdid = np.block([[e64, z64], [z64, e64]]).astype(np.float16)

    img16 = img.astype(np.float16)
    nc = _get_nc()
    in_maps = [
        {"img": img16[k * BSH:(k + 1) * BSH], "rhs01": rhs01, "rhs2": rhs2,
         "bdid": bdid, "invq": invq768}
        for k in range(NCORES)
    ]
    res = run_bass_kernel_spmd(nc, in_maps, core_ids=list(range(NCORES)))
    flatten = np.concatenate(
        [_unshard(np.asarray(r["qqd"])) for r in res.results], axis=1)
    no_quan = np.concatenate(
        [_unshard(np.asarray(r["nqd"])) for r in res.results], axis=1)
    return (flatten, no_quan)
